# revision 2
# baseline (speedup 1.0000x reference)
"""MoE expert-parallel kernel for Trainium2 (8 NeuronCores), load-balanced.

Problem: nn_Experts (T=8192 tokens, d_model=1024, d_ff=4096, E=8 experts,
top-k=2).  out[t] = sum_e w[t,e] * (relu(x[t] @ wi[e].T) @ wo[e].T), where
w[t,e] is the combined routing weight (0 for unrouted pairs).

Strategy (expert parallelism with quarter-ff load balancing):
  - Host: compute w[t,e] and per-expert token lists.  Each expert's
    d_ff=4096 is split into 4 quarters of 1024, placed on 4 different
    cores.  Cores hold 4 slots; slot k serves the rank-2k (cores 0-3) or
    rank-2k+1 (cores 4-7) expert by routed-token count (descending), so
    per-core capacity is sum_k count(rank 2k)/4 ~= the mean expert load
    (~1930 token-equivalents) instead of the max (~1992): ~3% less work
    than one-expert-per-core.
  - Device (per core, SPMD): for each slot and token chunk (width <= 504;
    N=512 exactly hits a PSUM-bank pathology ~30% slower per matmul):
    h = relu(wi_q @ xT) over 8x8 [128,128] weight tiles, partial
    yT = wo_q.T-slice @ h over 8x8 tiles.  Weights bf16 resident in SBUF
    (128 KiB/partition), fp32 PSUM accumulation, x streamed per chunk.
  - Host: sum the 4 fp32 partial-y per expert, scale by w[t,e],
    scatter-add into the full [T, d_model] output.

Measured numerics (vs fp32 reference): max-abs rel err ~3.7e-3.
"""
import os
import sys
from contextlib import ExitStack

import numpy as np

sys.path.insert(0, "/opt/trn_rl_repo")

import concourse.bass as bass
import concourse.mybir as mybir
from concourse import tile
from concourse import bass2jax
from concourse.bass2jax import _bass_exec_p, install_neuronx_cc_hook

T, D_MODEL, D_FF, N_EXPERTS, TOP_K = 8192, 1024, 4096, 8, 2
N_CORES = 8
P = 128          # partitions
TC = 504         # max chunk width (<=504: one fp32 PSUM bank, and N=512
                 # exactly measures ~30% slower per matmul on this HW)
KD = D_MODEL // P    # 8 contraction tiles for mm1 / output tiles for mm2
QF = D_FF // 4       # 1024: quarter of d_ff
MQ = QF // P         # 8 ff tiles per quarter
COMPUTE_DT = mybir.dt.bfloat16


def split_multi_waits(nc, max_waits=1):
    """This container's walrus codegen rejects instructions carrying more
    than a couple of semaphore waits (e.g. the TileContext tail Drain).
    Move excess waits onto preceding NoOps on the same engine."""
    for f in nc.m.functions:
        for b in f.blocks:
            il = b.instructions
            i = 0
            while i < len(il):
                inst = il[i]
                si = inst.sync_info
                if si is not None and len(si.on_wait) > max_waits:
                    waits = list(si.on_wait)
                    si.on_wait = waits[:max_waits]
                    inst.sync_info = si
                    pre = []
                    rest = waits[max_waits:]
                    for k in range(0, len(rest), max_waits):
                        nop = mybir.InstNoOp(
                            name=f"{inst.name}-ws-{k}", ins=[], outs=[])
                        nop.engine = inst.engine
                        nop.sync_info = mybir.SyncInfo(
                            on_wait=rest[k:k + max_waits], on_update=[])
                        pre.append(nop)
                    for n in reversed(pre):
                        il.insert(i, n)
                    i += len(pre)
                i += 1


class SpmdRunner:
    """Compile a Bass program once; run it SPMD on n_cores via PJRT/axon."""

    def __init__(self, nc, n_cores):
        import jax
        from jax.sharding import Mesh, PartitionSpec
        from jax.experimental.shard_map import shard_map

        install_neuronx_cc_hook()
        self.nc = nc
        self.n_cores = n_cores
        partition_name = (nc.partition_id_tensor.name
                          if nc.partition_id_tensor else None)
        in_names, out_names, out_avals, zero_outs = [], [], [], []
        for alloc in nc.m.functions[0].allocations:
            if not isinstance(alloc, mybir.MemoryLocationSet):
                continue
            name = alloc.memorylocations[0].name
            if alloc.kind == "ExternalInput":
                if name != partition_name:
                    in_names.append(name)
            elif alloc.kind == "ExternalOutput":
                out_names.append(name)
                shape = tuple(alloc.tensor_shape)
                dtype = mybir.dt.np(alloc.dtype)
                out_avals.append(jax.core.ShapedArray(shape, dtype))
                zero_outs.append(np.zeros(shape, dtype))
        self.in_names = in_names
        self.out_names = out_names
        self.out_avals = out_avals
        self.zero_outs = zero_outs
        n_params = len(in_names)
        n_outs = len(out_avals)
        all_in_names = list(in_names) + list(out_names)
        if partition_name is not None:
            all_in_names.append(partition_name)
        donate = tuple(range(n_params, n_params + n_outs))

        def _body(*args):
            operands = list(args)
            if partition_name is not None:
                operands.append(bass2jax.partition_id_tensor())
            outs = _bass_exec_p.bind(
                *operands,
                out_avals=tuple(out_avals),
                in_names=tuple(all_in_names),
                out_names=tuple(out_names),
                lowering_input_output_aliases=(),
                sim_require_finite=True,
                sim_require_nnan=True,
                nc=nc,
            )
            return tuple(outs)

        devices = jax.devices()[:n_cores]
        assert len(devices) == n_cores, (
            f"need {n_cores} neuron cores, found {len(jax.devices())}")
        mesh = Mesh(np.asarray(devices), ("core",))
        self.mesh = mesh
        in_specs = (PartitionSpec("core"),) * (n_params + n_outs)
        out_specs = (PartitionSpec("core"),) * n_outs
        self.sharded = jax.jit(
            shard_map(_body, mesh=mesh, in_specs=in_specs,
                      out_specs=out_specs, check_rep=False),
            donate_argnums=donate, keep_unused=True)

    def prep(self, in_maps):
        n = self.n_cores
        concat_in = [
            np.concatenate([np.asarray(in_maps[c][name]) for c in range(n)],
                           axis=0)
            for name in self.in_names
        ]
        concat_zeros = self.device_zeros()
        return concat_in, concat_zeros

    def device_zeros(self):
        """Donated output buffers, created directly on device (no H2D)."""
        import jax
        import jax.numpy as jnp
        from jax.sharding import NamedSharding, PartitionSpec
        if not hasattr(self, "_zeros_fn"):
            n = self.n_cores
            shapes = [(n * z.shape[0], *z.shape[1:]) for z in self.zero_outs]
            dts = [z.dtype for z in self.zero_outs]
            sh = tuple(NamedSharding(self.mesh, PartitionSpec("core"))
                       for _ in shapes)
            self._zeros_fn = jax.jit(
                lambda: tuple(jnp.zeros(s, d) for s, d in zip(shapes, dts)),
                out_shardings=sh)
        return list(self._zeros_fn())

    def run_prepped(self, concat_in, concat_zeros=None):
        if concat_zeros is None:
            concat_zeros = self.device_zeros()
        return self.sharded(*concat_in, *concat_zeros)


def chunk_widths(C):
    """Split C tokens into near-equal chunks of width <= TC (multiple of 8)."""
    n = -(-C // TC)
    w = -(-(-(-C // n)) // 8) * 8
    widths = [w] * (n - 1) + [C - w * (n - 1)]
    assert all(0 < x <= TC for x in widths) and sum(widths) == C, (C, widths)
    return widths


def build_nc(Cs, n_repeat=1):
    """Per-core program for 4 slots with token capacities Cs[0..3].

    Inputs per core:
      x{s}:  [nchunk_s * D_MODEL, TCW_s] bf16, chunk-major packed tokens
      wiT:   [4*D_MODEL, QF]  bf16  (slot-major; wi quarter, transposed)
      woT:   [4*QF, D_MODEL]  bf16  (slot-major; wo quarter, transposed)
    Outputs per core:
      y{s}:  [D_MODEL, C_s] fp32   (partial y for this quarter of d_ff)

    n_repeat>1 wraps the sweep in a hardware loop (for slope timing; the
    result is identical each iteration)."""
    nc = bass.Bass()
    xps, yps = [], []
    slot_widths = []
    for s, C in enumerate(Cs):
        widths = chunk_widths(C)
        slot_widths.append(widths)
        xps.append(nc.declare_dram_parameter(
            f"x{s}", [len(widths) * D_MODEL, widths[0]], COMPUTE_DT,
            isOutput=False))
        yps.append(nc.declare_dram_parameter(
            f"y{s}", [D_MODEL, C], mybir.dt.float32, isOutput=True))
    wiT = nc.declare_dram_parameter("wiT", [4 * D_MODEL, QF], COMPUTE_DT,
                                    isOutput=False)
    woT = nc.declare_dram_parameter("woT", [4 * QF, D_MODEL], COMPUTE_DT,
                                    isOutput=False)

    with ExitStack() as ctx:
        tc = ctx.enter_context(tile.TileContext(nc))
        wpool = ctx.enter_context(tc.tile_pool(name="w", bufs=1))
        xpool = ctx.enter_context(tc.tile_pool(name="x", bufs=2))
        hpool = ctx.enter_context(tc.tile_pool(name="h", bufs=2))
        ypool = ctx.enter_context(tc.tile_pool(name="y", bufs=4))
        pspool = ctx.enter_context(
            tc.tile_pool(name="ps", bufs=8, space="PSUM"))

        # x for slot 0 chunk 0 first so the first matmuls start early.
        def x_dma(t, s, c, k, w):
            r0 = c * D_MODEL + k * P
            nc.sync.dma_start(out=t[:], in_=xps[s][r0:r0 + P, 0:w])

        x0_t = []
        for k in range(KD):
            t = xpool.tile([P, slot_widths[0][0]], COMPUTE_DT, tag=f"x{k}")
            x_dma(t, 0, 0, k, slot_widths[0][0])
            x0_t.append(t)

        wi_t, wo_t = [], []
        for s in range(4):
            row = []
            for k in range(KD):
                t = wpool.tile([P, QF], COMPUTE_DT, tag=f"wi{s}_{k}")
                nc.sync.dma_start(
                    out=t[:],
                    in_=wiT[s * D_MODEL + k * P:s * D_MODEL + (k + 1) * P, :])
                row.append(t)
            wi_t.append(row)
            row = []
            for m in range(MQ):
                t = wpool.tile([P, D_MODEL], COMPUTE_DT, tag=f"wo{s}_{m}")
                nc.sync.dma_start(
                    out=t[:],
                    in_=woT[s * QF + m * P:s * QF + (m + 1) * P, :])
                row.append(t)
            wo_t.append(row)

        def sweep(first=False):
            for s in range(4):
                widths = slot_widths[s]
                starts = [sum(widths[:i]) for i in range(len(widths))]
                for c, (c0, w) in enumerate(zip(starts, widths)):
                    if first and s == 0 and c == 0:
                        x_t = x0_t
                    else:
                        x_t = []
                        for k in range(KD):
                            t = xpool.tile([P, w], COMPUTE_DT, tag=f"x{k}")
                            x_dma(t, s, c, k, w)
                            x_t.append(t)
                    h_t = []
                    for m in range(MQ):
                        ps = pspool.tile([P, w], mybir.dt.float32, tag="ps")
                        for k in range(KD):
                            nc.tensor.matmul(
                                ps[:], wi_t[s][k][:, m * P:(m + 1) * P],
                                x_t[k][:], start=(k == 0), stop=(k == KD - 1))
                        h = hpool.tile([P, w], COMPUTE_DT, tag=f"h{m}")
                        nc.scalar.activation(
                            h[:], ps[:], mybir.ActivationFunctionType.Relu)
                        h_t.append(h)
                    for n in range(KD):
                        ps = pspool.tile([P, w], mybir.dt.float32, tag="ps")
                        for m in range(MQ):
                            nc.tensor.matmul(
                                ps[:], wo_t[s][m][:, n * P:(n + 1) * P],
                                h_t[m][:], start=(m == 0), stop=(m == MQ - 1))
                        y = ypool.tile([P, w], mybir.dt.float32, tag="y")
                        nc.vector.tensor_copy(y[:], ps[:])
                        nc.sync.dma_start(
                            out=yps[s][n * P:(n + 1) * P, c0:c0 + w],
                            in_=y[:])

        if n_repeat == 1:
            sweep(first=True)
        else:
            with tc.For_i(0, n_repeat, 1,
                          hint_engines=(mybir.EngineType.PE,)):
                sweep()

    split_multi_waits(nc)
    return nc


_RUNNERS = {}


def _get_runner(Cs, n_repeat=1):
    key = (tuple(Cs), n_repeat)
    if key not in _RUNNERS:
        _RUNNERS[key] = SpmdRunner(build_nc(list(Cs), n_repeat), N_CORES)
    return _RUNNERS[key]


def _route(hidden_states, selected_experts, routing_weights):
    """Combined per-token weight for each expert and per-expert token lists."""
    mask = selected_experts.astype(np.float32)          # [T, K, E]
    w_te = np.einsum('tke,tk->te', mask, routing_weights.astype(np.float32))
    idx = [np.nonzero(w_te[:, e] > 0)[0] for e in range(N_EXPERTS)]
    return w_te, idx


def plan(counts):
    """Rank experts by count desc; slot k serves experts rank 2k (cores 0-3)
    and rank 2k+1 (cores 4-7).  Returns (order, Cs): order[k] = (expert on
    cores 0-3, expert on cores 4-7); Cs[k] = padded slot capacity."""
    order_desc = np.argsort(-np.asarray(counts), kind="stable")
    order = [(int(order_desc[2 * k]), int(order_desc[2 * k + 1]))
             for k in range(4)]
    Cs = [max(8, ((int(counts[a]) + 7) // 8) * 8) for a, _ in order]
    return order, Cs


def to_bf16(a):
    """Vectorized fp32 -> bf16 cast (round-to-nearest-even), ~3x faster
    than ml_dtypes astype."""
    import ml_dtypes
    a = np.ascontiguousarray(a, dtype=np.float32)
    u = a.view(np.uint32)
    r = ((u + 0x7FFF + ((u >> 16) & 1)) >> 16).astype(np.uint16)
    return r.view(ml_dtypes.bfloat16).reshape(a.shape)


def pack_x_slot(hidden_states, ie, C):
    """Chunk-major packed xT for one slot: [nchunk*D_MODEL, TCW] bf16, so
    every per-tile DMA in the kernel is a contiguous block."""
    import ml_dtypes
    widths = chunk_widths(C)
    TCW = widths[0]
    xg = to_bf16(hidden_states[ie].transpose(1, 0))      # [D_MODEL, n]
    out = np.zeros((len(widths) * D_MODEL, TCW), dtype=ml_dtypes.bfloat16)
    c0 = 0
    for c, w in enumerate(widths):
        seg = xg[:, c0:min(c0 + w, xg.shape[1])]
        out[c * D_MODEL:(c + 1) * D_MODEL, :seg.shape[1]] = seg
        c0 += w
    return out


def make_in_maps(hidden_states, idx, wi, wo, order, Cs):
    """Per-core input dicts."""
    in_maps = []
    for j in range(N_CORES):
        q = j % 4
        side = j // 4
        wi_rows, wo_rows = [], []
        xmap = {}
        for k in range(4):
            e = order[k][side]
            wi_q = wi[e][q * QF:(q + 1) * QF, :]     # [QF, D_MODEL]
            wo_q = wo[e][:, q * QF:(q + 1) * QF]     # [D_MODEL, QF]
            wi_rows.append(to_bf16(np.ascontiguousarray(wi_q.T)))
            wo_rows.append(to_bf16(np.ascontiguousarray(wo_q.T)))
            xmap[f"x{k}"] = pack_x_slot(hidden_states, idx[e], Cs[k])
        xmap["wiT"] = np.concatenate(wi_rows, axis=0)
        xmap["woT"] = np.concatenate(wo_rows, axis=0)
        in_maps.append(xmap)
    return in_maps


def kernel(hidden_states, selected_experts, routing_weights, wi, wo):
    hidden_states = np.asarray(hidden_states)
    selected_experts = np.asarray(selected_experts)
    routing_weights = np.asarray(routing_weights)
    wi = np.asarray(wi)
    wo = np.asarray(wo)

    w_te, idx = _route(hidden_states, selected_experts, routing_weights)
    counts = [len(i) for i in idx]
    order, Cs = plan(counts)
    runner = _get_runner(Cs)

    in_maps = make_in_maps(hidden_states, idx, wi, wo, order, Cs)
    out_arrs = runner.run_prepped(runner.prep(in_maps)[0])

    out = np.zeros((T, D_MODEL), dtype=np.float32)
    name_to_arr = dict(zip(runner.out_names, out_arrs))
    for k in range(4):
        yk = np.asarray(name_to_arr[f"y{k}"]).reshape(
            N_CORES, D_MODEL, Cs[k])
        for side in range(2):
            e = order[k][side]
            ie = idx[e]
            ysum = yk[side * 4:side * 4 + 4, :, :len(ie)].sum(axis=0)
            out[ie] += w_te[ie, e:e + 1] * ysum.T
    return out


# revision 3
# speedup vs baseline: 1.0178x; 1.0178x over previous
"""MoE expert-parallel kernel for Trainium2 (8 NeuronCores), load-balanced.

Problem: nn_Experts (T=8192 tokens, d_model=1024, d_ff=4096, E=8 experts,
top-k=2).  out[t] = sum_e w[t,e] * (relu(x[t] @ wi[e].T) @ wo[e].T), where
w[t,e] is the combined routing weight (0 for unrouted pairs).

Strategy (expert parallelism with quarter-ff load balancing):
  - Host: compute w[t,e] and per-expert token lists.  Each expert's
    d_ff=4096 is split into 4 quarters of 1024, placed on 4 different
    cores.  Cores hold 4 slots; slot k serves the rank-2k (cores 0-3) or
    rank-2k+1 (cores 4-7) expert by routed-token count (descending), so
    per-core capacity is sum_k count(rank 2k)/4 ~= the mean expert load
    (~1930 token-equivalents) instead of the max (~1992): ~3% less work
    than one-expert-per-core.
  - Device (per core, SPMD): for each slot and token chunk (width <= 504;
    N=512 exactly hits a PSUM-bank pathology ~30% slower per matmul):
    h = relu(wi_q @ xT) over 8x8 [128,128] weight tiles, partial
    yT = wo_q.T-slice @ h over 8x8 tiles.  Weights bf16 resident in SBUF
    (128 KiB/partition), fp32 PSUM accumulation, x streamed per chunk.
  - Host: sum the 4 fp32 partial-y per expert, scale by w[t,e],
    scatter-add into the full [T, d_model] output.

Measured numerics (vs fp32 reference): max-abs rel err ~3.7e-3.
"""
import os
import sys
from contextlib import ExitStack

import numpy as np

sys.path.insert(0, "/opt/trn_rl_repo")

import concourse.bass as bass
import concourse.mybir as mybir
from concourse import tile
from concourse import bass2jax
from concourse.bass2jax import _bass_exec_p, install_neuronx_cc_hook

T, D_MODEL, D_FF, N_EXPERTS, TOP_K = 8192, 1024, 4096, 8, 2
N_CORES = 8
P = 128          # partitions
TC = 456         # max chunk width; must stay <=504 (one fp32 PSUM bank;
                 # N=512 exactly measures ~30% slower per matmul) and 456
                 # beat 504 by ~1% in interleaved A/B on this HW
KD = D_MODEL // P    # 8 contraction tiles for mm1 / output tiles for mm2
QF = D_FF // 4       # 1024: quarter of d_ff
MQ = QF // P         # 8 ff tiles per quarter
COMPUTE_DT = mybir.dt.bfloat16


def split_multi_waits(nc, max_waits=1):
    """This container's walrus codegen rejects instructions carrying more
    than a couple of semaphore waits (e.g. the TileContext tail Drain).
    Move excess waits onto preceding NoOps on the same engine."""
    for f in nc.m.functions:
        for b in f.blocks:
            il = b.instructions
            i = 0
            while i < len(il):
                inst = il[i]
                si = inst.sync_info
                if si is not None and len(si.on_wait) > max_waits:
                    waits = list(si.on_wait)
                    si.on_wait = waits[:max_waits]
                    inst.sync_info = si
                    pre = []
                    rest = waits[max_waits:]
                    for k in range(0, len(rest), max_waits):
                        nop = mybir.InstNoOp(
                            name=f"{inst.name}-ws-{k}", ins=[], outs=[])
                        nop.engine = inst.engine
                        nop.sync_info = mybir.SyncInfo(
                            on_wait=rest[k:k + max_waits], on_update=[])
                        pre.append(nop)
                    for n in reversed(pre):
                        il.insert(i, n)
                    i += len(pre)
                i += 1


class SpmdRunner:
    """Compile a Bass program once; run it SPMD on n_cores via PJRT/axon."""

    def __init__(self, nc, n_cores):
        import jax
        from jax.sharding import Mesh, PartitionSpec
        from jax.experimental.shard_map import shard_map

        install_neuronx_cc_hook()
        self.nc = nc
        self.n_cores = n_cores
        partition_name = (nc.partition_id_tensor.name
                          if nc.partition_id_tensor else None)
        in_names, out_names, out_avals, zero_outs = [], [], [], []
        for alloc in nc.m.functions[0].allocations:
            if not isinstance(alloc, mybir.MemoryLocationSet):
                continue
            name = alloc.memorylocations[0].name
            if alloc.kind == "ExternalInput":
                if name != partition_name:
                    in_names.append(name)
            elif alloc.kind == "ExternalOutput":
                out_names.append(name)
                shape = tuple(alloc.tensor_shape)
                dtype = mybir.dt.np(alloc.dtype)
                out_avals.append(jax.core.ShapedArray(shape, dtype))
                zero_outs.append(np.zeros(shape, dtype))
        self.in_names = in_names
        self.out_names = out_names
        self.out_avals = out_avals
        self.zero_outs = zero_outs
        n_params = len(in_names)
        n_outs = len(out_avals)
        all_in_names = list(in_names) + list(out_names)
        if partition_name is not None:
            all_in_names.append(partition_name)
        donate = tuple(range(n_params, n_params + n_outs))

        def _body(*args):
            operands = list(args)
            if partition_name is not None:
                operands.append(bass2jax.partition_id_tensor())
            outs = _bass_exec_p.bind(
                *operands,
                out_avals=tuple(out_avals),
                in_names=tuple(all_in_names),
                out_names=tuple(out_names),
                lowering_input_output_aliases=(),
                sim_require_finite=True,
                sim_require_nnan=True,
                nc=nc,
            )
            return tuple(outs)

        devices = jax.devices()[:n_cores]
        assert len(devices) == n_cores, (
            f"need {n_cores} neuron cores, found {len(jax.devices())}")
        mesh = Mesh(np.asarray(devices), ("core",))
        self.mesh = mesh
        in_specs = (PartitionSpec("core"),) * (n_params + n_outs)
        out_specs = (PartitionSpec("core"),) * n_outs
        self.sharded = jax.jit(
            shard_map(_body, mesh=mesh, in_specs=in_specs,
                      out_specs=out_specs, check_rep=False),
            donate_argnums=donate, keep_unused=True)

    def prep(self, in_maps):
        n = self.n_cores
        concat_in = [
            np.concatenate([np.asarray(in_maps[c][name]) for c in range(n)],
                           axis=0)
            for name in self.in_names
        ]
        concat_zeros = self.device_zeros()
        return concat_in, concat_zeros

    def device_zeros(self):
        """Donated output buffers, created directly on device (no H2D)."""
        import jax
        import jax.numpy as jnp
        from jax.sharding import NamedSharding, PartitionSpec
        if not hasattr(self, "_zeros_fn"):
            n = self.n_cores
            shapes = [(n * z.shape[0], *z.shape[1:]) for z in self.zero_outs]
            dts = [z.dtype for z in self.zero_outs]
            sh = tuple(NamedSharding(self.mesh, PartitionSpec("core"))
                       for _ in shapes)
            self._zeros_fn = jax.jit(
                lambda: tuple(jnp.zeros(s, d) for s, d in zip(shapes, dts)),
                out_shardings=sh)
        return list(self._zeros_fn())

    def run_prepped(self, concat_in, concat_zeros=None):
        if concat_zeros is None:
            concat_zeros = self.device_zeros()
        return self.sharded(*concat_in, *concat_zeros)


def chunk_widths(C):
    """Split C tokens into near-equal chunks of width <= TC (multiple of 8)."""
    n = -(-C // TC)
    w = -(-(-(-C // n)) // 8) * 8
    widths = [w] * (n - 1) + [C - w * (n - 1)]
    assert all(0 < x <= TC for x in widths) and sum(widths) == C, (C, widths)
    return widths


def build_nc(Cs, n_repeat=1):
    """Per-core program for 4 slots with token capacities Cs[0..3].

    Inputs per core:
      x{s}:  [nchunk_s * D_MODEL, TCW_s] bf16, chunk-major packed tokens
      wiT:   [4*D_MODEL, QF]  bf16  (slot-major; wi quarter, transposed)
      woT:   [4*QF, D_MODEL]  bf16  (slot-major; wo quarter, transposed)
    Outputs per core:
      y{s}:  [D_MODEL, C_s] fp32   (partial y for this quarter of d_ff)

    n_repeat>1 wraps the sweep in a hardware loop (for slope timing; the
    result is identical each iteration)."""
    nc = bass.Bass()
    xps, yps = [], []
    slot_widths = []
    for s, C in enumerate(Cs):
        widths = chunk_widths(C)
        slot_widths.append(widths)
        xps.append(nc.declare_dram_parameter(
            f"x{s}", [len(widths) * D_MODEL, widths[0]], COMPUTE_DT,
            isOutput=False))
        yps.append(nc.declare_dram_parameter(
            f"y{s}", [D_MODEL, C], mybir.dt.float32, isOutput=True))
    wiT = nc.declare_dram_parameter("wiT", [4 * D_MODEL, QF], COMPUTE_DT,
                                    isOutput=False)
    woT = nc.declare_dram_parameter("woT", [4 * QF, D_MODEL], COMPUTE_DT,
                                    isOutput=False)

    with ExitStack() as ctx:
        tc = ctx.enter_context(tile.TileContext(nc))
        wpool = ctx.enter_context(tc.tile_pool(name="w", bufs=1))
        xpool = ctx.enter_context(tc.tile_pool(name="x", bufs=2))
        hpool = ctx.enter_context(tc.tile_pool(name="h", bufs=2))
        ypool = ctx.enter_context(tc.tile_pool(name="y", bufs=4))
        pspool = ctx.enter_context(
            tc.tile_pool(name="ps", bufs=8, space="PSUM"))

        # x for slot 0 chunk 0 first so the first matmuls start early.
        def x_dma(t, s, c, k, w):
            r0 = c * D_MODEL + k * P
            nc.sync.dma_start(out=t[:], in_=xps[s][r0:r0 + P, 0:w])

        x0_t = []
        for k in range(KD):
            t = xpool.tile([P, slot_widths[0][0]], COMPUTE_DT, tag=f"x{k}")
            x_dma(t, 0, 0, k, slot_widths[0][0])
            x0_t.append(t)

        wi_t, wo_t = [], []
        for s in range(4):
            row = []
            for k in range(KD):
                t = wpool.tile([P, QF], COMPUTE_DT, tag=f"wi{s}_{k}")
                nc.sync.dma_start(
                    out=t[:],
                    in_=wiT[s * D_MODEL + k * P:s * D_MODEL + (k + 1) * P, :])
                row.append(t)
            wi_t.append(row)
            row = []
            for m in range(MQ):
                t = wpool.tile([P, D_MODEL], COMPUTE_DT, tag=f"wo{s}_{m}")
                nc.sync.dma_start(
                    out=t[:],
                    in_=woT[s * QF + m * P:s * QF + (m + 1) * P, :])
                row.append(t)
            wo_t.append(row)

        def sweep(first=False):
            for s in range(4):
                widths = slot_widths[s]
                starts = [sum(widths[:i]) for i in range(len(widths))]
                for c, (c0, w) in enumerate(zip(starts, widths)):
                    if first and s == 0 and c == 0:
                        x_t = x0_t
                    else:
                        x_t = []
                        for k in range(KD):
                            t = xpool.tile([P, w], COMPUTE_DT, tag=f"x{k}")
                            x_dma(t, s, c, k, w)
                            x_t.append(t)
                    h_t = []
                    for m in range(MQ):
                        ps = pspool.tile([P, w], mybir.dt.float32, tag="ps")
                        for k in range(KD):
                            nc.tensor.matmul(
                                ps[:], wi_t[s][k][:, m * P:(m + 1) * P],
                                x_t[k][:], start=(k == 0), stop=(k == KD - 1))
                        h = hpool.tile([P, w], COMPUTE_DT, tag=f"h{m}")
                        nc.scalar.activation(
                            h[:], ps[:], mybir.ActivationFunctionType.Relu)
                        h_t.append(h)
                    for n in range(KD):
                        ps = pspool.tile([P, w], mybir.dt.float32, tag="ps")
                        for m in range(MQ):
                            nc.tensor.matmul(
                                ps[:], wo_t[s][m][:, n * P:(n + 1) * P],
                                h_t[m][:], start=(m == 0), stop=(m == MQ - 1))
                        y = ypool.tile([P, w], mybir.dt.float32, tag="y")
                        nc.vector.tensor_copy(y[:], ps[:])
                        nc.sync.dma_start(
                            out=yps[s][n * P:(n + 1) * P, c0:c0 + w],
                            in_=y[:])

        if n_repeat == 1:
            sweep(first=True)
        else:
            with tc.For_i(0, n_repeat, 1,
                          hint_engines=(mybir.EngineType.PE,)):
                sweep()

    split_multi_waits(nc)
    return nc


_RUNNERS = {}


def _get_runner(Cs, n_repeat=1):
    key = (tuple(Cs), n_repeat)
    if key not in _RUNNERS:
        _RUNNERS[key] = SpmdRunner(build_nc(list(Cs), n_repeat), N_CORES)
    return _RUNNERS[key]


def _route(hidden_states, selected_experts, routing_weights):
    """Combined per-token weight for each expert and per-expert token lists."""
    mask = selected_experts.astype(np.float32)          # [T, K, E]
    w_te = np.einsum('tke,tk->te', mask, routing_weights.astype(np.float32))
    idx = [np.nonzero(w_te[:, e] > 0)[0] for e in range(N_EXPERTS)]
    return w_te, idx


def plan(counts):
    """Rank experts by count desc; slot k serves experts rank 2k (cores 0-3)
    and rank 2k+1 (cores 4-7).  Returns (order, Cs): order[k] = (expert on
    cores 0-3, expert on cores 4-7); Cs[k] = padded slot capacity."""
    order_desc = np.argsort(-np.asarray(counts), kind="stable")
    order = [(int(order_desc[2 * k]), int(order_desc[2 * k + 1]))
             for k in range(4)]
    Cs = [max(8, ((int(counts[a]) + 7) // 8) * 8) for a, _ in order]
    return order, Cs


def to_bf16(a):
    """Vectorized fp32 -> bf16 cast (round-to-nearest-even), ~3x faster
    than ml_dtypes astype."""
    import ml_dtypes
    a = np.ascontiguousarray(a, dtype=np.float32)
    u = a.view(np.uint32)
    r = ((u + 0x7FFF + ((u >> 16) & 1)) >> 16).astype(np.uint16)
    return r.view(ml_dtypes.bfloat16).reshape(a.shape)


def pack_x_slot(hidden_states, ie, C):
    """Chunk-major packed xT for one slot: [nchunk*D_MODEL, TCW] bf16, so
    every per-tile DMA in the kernel is a contiguous block."""
    import ml_dtypes
    widths = chunk_widths(C)
    TCW = widths[0]
    xg = to_bf16(hidden_states[ie].transpose(1, 0))      # [D_MODEL, n]
    out = np.zeros((len(widths) * D_MODEL, TCW), dtype=ml_dtypes.bfloat16)
    c0 = 0
    for c, w in enumerate(widths):
        seg = xg[:, c0:min(c0 + w, xg.shape[1])]
        out[c * D_MODEL:(c + 1) * D_MODEL, :seg.shape[1]] = seg
        c0 += w
    return out


def make_in_maps(hidden_states, idx, wi, wo, order, Cs):
    """Per-core input dicts."""
    in_maps = []
    for j in range(N_CORES):
        q = j % 4
        side = j // 4
        wi_rows, wo_rows = [], []
        xmap = {}
        for k in range(4):
            e = order[k][side]
            wi_q = wi[e][q * QF:(q + 1) * QF, :]     # [QF, D_MODEL]
            wo_q = wo[e][:, q * QF:(q + 1) * QF]     # [D_MODEL, QF]
            wi_rows.append(to_bf16(np.ascontiguousarray(wi_q.T)))
            wo_rows.append(to_bf16(np.ascontiguousarray(wo_q.T)))
            xmap[f"x{k}"] = pack_x_slot(hidden_states, idx[e], Cs[k])
        xmap["wiT"] = np.concatenate(wi_rows, axis=0)
        xmap["woT"] = np.concatenate(wo_rows, axis=0)
        in_maps.append(xmap)
    return in_maps


def kernel(hidden_states, selected_experts, routing_weights, wi, wo):
    hidden_states = np.asarray(hidden_states)
    selected_experts = np.asarray(selected_experts)
    routing_weights = np.asarray(routing_weights)
    wi = np.asarray(wi)
    wo = np.asarray(wo)

    w_te, idx = _route(hidden_states, selected_experts, routing_weights)
    counts = [len(i) for i in idx]
    order, Cs = plan(counts)
    runner = _get_runner(Cs)

    in_maps = make_in_maps(hidden_states, idx, wi, wo, order, Cs)
    out_arrs = runner.run_prepped(runner.prep(in_maps)[0])

    out = np.zeros((T, D_MODEL), dtype=np.float32)
    name_to_arr = dict(zip(runner.out_names, out_arrs))
    for k in range(4):
        yk = np.asarray(name_to_arr[f"y{k}"]).reshape(
            N_CORES, D_MODEL, Cs[k])
        for side in range(2):
            e = order[k][side]
            ie = idx[e]
            ysum = yk[side * 4:side * 4 + 4, :, :len(ie)].sum(axis=0)
            out[ie] += w_te[ie, e:e + 1] * ysum.T
    return out
